# revision 4
# baseline (speedup 1.0000x reference)
"""Sparse (DAG-masked) attention head on 8 Trainium2 NeuronCores.

Reference computation (per batch b of 64):
    K = X_b @ Wk + bk; Q = Y_b @ Wq + bq; V = X_b @ Wv + bv         [T=1024, H=512]
    S = Q @ K^T / sqrt(H); A = softmax(where(dag.T*S == 0, -inf, dag.T*S))
    O = A @ V   (fully-masked rows -> 0)

Strategy: data-parallel over batch (8 batches per core); weights + dag
replicated. All matmuls run in float32r (TF32-like, 1 cycle/row on PE,
~1e-4 relative error).

Key algebraic fusion: softmax over s is invariant to additive terms that
vary only in t, so
    S^T[s,t] = (X G Y^T)[s,t] + beta[s] + (t-only terms, cancel)
with G = Wk @ Wq^T and beta = X @ (Wk @ bq), both folded on the host.
This removes one of the three projections and both K/Q bias adds.

Scores are computed TRANSPOSED (ST[s,t] = Z @ Y^T with Z^T = G^T X^T) so
the softmax weights PT = dag * exp(ST/sqrt(H) + beta*scale) land directly
in the [s, t] layout needed as the stationary operand of the P @ V
matmul -- no on-chip transposes. Softmax skips max-subtraction (scores
are ~N(0,1); exp cannot overflow) and normalizes AFTER the V-matmul.

Two "free column" fusions keep every PE row a productive row:
  * beta[s] = X @ (Wk bq) is column 512 of the V projection: the rhs is
    [Wv | g] (514 wide, split 256+258 across two PSUM banks; f32r needs even free sizes, so the extra column is duplicated), so beta
    lands per-partition with no extra matmul and no scatter DMAs.
  * l[t] = sum_s PT[s,t] is column 512 of the U matmul: V is extended
    with a ones column, so the softmax denominator falls out of U's
    second PSUM bank already in per-partition layout.

Host-side prep: X/Y are transposed to [D, T] per batch (the PE contracts
over the partition dim).
"""

import numpy as np

import concourse.bass as bass
import concourse.mybir as mybir
import concourse.tile as tile
from concourse import bacc
from concourse.bass_utils import run_bass_kernel_spmd

B, T, D, H = 64, 1024, 512, 512
NCORES = 8
BPC = B // NCORES          # batches per core
DC = D // 128              # d chunks (4)
TC = T // 128              # t/s tiles (8)
SCALE = 1.0 / float(np.sqrt(H))

f32 = mybir.dt.float32
f32r = mybir.dt.float32r
bf16 = mybir.dt.bfloat16
f16 = mybir.dt.float16
EXP = mybir.ActivationFunctionType.Exp
COPY = mybir.ActivationFunctionType.Copy

_CACHED_NC = None

MM_DT = f32r               # matmul operand dtype: f32r (accurate) or bf16


def _build(reps=1, mm_dt=None):
    # reps>1 wraps the whole pipeline in a hardware loop that re-runs it on
    # the same data -- used only by the timing harness (wall-clock deltas
    # cancel the axon RPC overhead).
    dt = MM_DT if mm_dt is None else mm_dt
    nc = bacc.Bacc("TRN2", target_bir_lowering=False, debug=False,
                   num_devices=NCORES)

    XTd = nc.dram_tensor("XT", [BPC, DC, 128, T], dt, kind="ExternalInput").ap()
    YTd = nc.dram_tensor("YT", [BPC, DC, 128, T], dt, kind="ExternalInput").ap()
    DAGd = nc.dram_tensor("dagr", [TC, 128, T], bf16, kind="ExternalInput").ap()
    Gd = nc.dram_tensor("Gr", [DC, 128, D], dt, kind="ExternalInput").ap()
    Wvd = nc.dram_tensor("Wvr", [DC, 128, H + 2], dt, kind="ExternalInput").ap()
    Bvd = nc.dram_tensor("bvb", [128, H], f32, kind="ExternalInput").ap()
    Od = nc.dram_tensor("O", [BPC, T, H], f32, kind="ExternalOutput").ap()

    with tile.TileContext(nc) as tc:
        with (
            tc.tile_pool(name="const", bufs=1) as const,
            tc.tile_pool(name="data", bufs=1) as data,
            tc.tile_pool(name="data2", bufs=2) as data2,
            tc.tile_pool(name="pipe", bufs=2) as pipe,
            tc.tile_pool(name="small", bufs=3) as small,
            tc.tile_pool(name="psum3", bufs=4, space="PSUM") as psum3,
            tc.tile_pool(name="psa", bufs=2, space="PSUM") as psa,
            tc.tile_pool(name="psb", bufs=2, space="PSUM") as psb,
        ):
            # ---- resident tensors ----
            # Input streams split across the three DMA-capable queues
            # (SP / ACT / GPSIMD); batch-0 activations interleaved with G in
            # consumption order so the first matmul starts after ~1MB of DMA.
            gt = const.tile([128, DC, D], dt, tag="gt")
            wv = const.tile([128, DC, H + 2], dt, tag="wv")
            bvb = const.tile([128, H], f32, tag="bvb")
            dag = const.tile([128, TC, T], bf16, tag="dag")
            if reps == 1:
                xt0 = data2.tile([128, DC, T], dt, tag="xt")
                yt0 = data.tile([128, DC, T], dt, tag="yt")
                for c in range(DC):
                    nc.sync.dma_start(out=gt[:, c], in_=Gd[c])
                    nc.sync.dma_start(out=xt0[:, c], in_=XTd[0, c])
                for c in range(DC):
                    nc.scalar.dma_start(out=yt0[:, c], in_=YTd[0, c])
            else:
                xt0 = yt0 = None
                for c in range(DC):
                    nc.sync.dma_start(out=gt[:, c], in_=Gd[c])
            for c in range(DC):
                nc.gpsimd.dma_start(out=wv[:, c], in_=Wvd[c])
            nc.gpsimd.dma_start(out=bvb[:], in_=Bvd[:])
            for i in range(TC):
                nc.gpsimd.dma_start(out=dag[:, i], in_=DAGd[i])

            def emit_batch(b):
                # ---- load activations (transposed: [d, t]) ----
                # xt (used by ZT/V/beta, early) on SP, double-buffered;
                # yt (used by scores, later) on ACT.
                if b == 0 and xt0 is not None:
                    xt, yt = xt0, yt0
                else:
                    xt = data2.tile([128, DC, T], dt, tag="xt")
                    yt = data.tile([128, DC, T], dt, tag="yt")
                    for c in range(DC):
                        nc.sync.dma_start(out=xt[:, c], in_=XTd[b, c])
                        nc.scalar.dma_start(out=yt[:, c], in_=YTd[b, c])

                # ---- ZT[d', s] = G^T X^T: lhsT = G[d, d'_tile], rhs = XT ----
                zt = data.tile([128, DC, T], dt, tag="zt")
                for j in range(DC):
                    for hf in range(2):
                        ps = psum3.tile([128, 512], f32, tag="mm")
                        for c in range(DC):
                            nc.tensor.matmul(
                                ps[:],
                                gt[:, c, j * 128:(j + 1) * 128],
                                xt[:, c, hf * 512:(hf + 1) * 512],
                                start=(c == 0), stop=(c == DC - 1),
                            )
                        nc.scalar.activation(
                            zt[:, j, hf * 512:(hf + 1) * 512], ps[:],
                            COPY, bias=0.0, scale=1.0,
                        )
                # ---- V[s, h] (plus beta column, plus a ones column for l):
                # lhsT = XT[d, s_tile], rhs = [Wv | g] 513 wide split over two
                # PSUM banks; col 512 accumulates beta[s] = X @ (Wk bq).
                v = data.tile([128, TC, H + 2], dt, tag="v")
                beta = small.tile([128, TC], f32, tag="beta")
                for i in range(TC):
                    pa = psa.tile([128, 256], f32, tag="pa")
                    pb = psb.tile([128, 258], f32, tag="pb")
                    for c in range(DC):
                        lhsT = xt[:, c, i * 128:(i + 1) * 128]
                        nc.tensor.matmul(
                            pb[:], lhsT, wv[:, c, 256:514],
                            start=(c == 0), stop=(c == DC - 1),
                        )
                        nc.tensor.matmul(
                            pa[:], lhsT, wv[:, c, 0:256],
                            start=(c == 0), stop=(c == DC - 1),
                        )
                    nc.vector.tensor_add(v[:, i, 0:256], pa[:], bvb[:, 0:256])
                    nc.vector.tensor_add(v[:, i, 256:512], pb[:, 0:256],
                                         bvb[:, 256:512])
                    nc.vector.tensor_scalar_mul(
                        beta[:, i:i + 1], pb[:, 256:257], SCALE)
                    # ones column for the l fold in the U matmul
                    nc.scalar.activation(v[:, i, 512:514], pb[:, 256:258],
                                         COPY, bias=1.0, scale=0.0)

                # ---- scores + AV in two t-halves (AV of one half overlaps
                # the score matmuls of the next).
                for th in range(2):
                    t0 = th * 512
                    # PT[s, t] = dag * exp(ST*scale + beta), ST = Z @ Y^T
                    pt = pipe.tile([128, TC, 512], dt, tag="pt")
                    for i in range(TC):
                        ps = psum3.tile([128, 512], f32, tag="mm")
                        for j in range(DC):
                            nc.tensor.matmul(
                                ps[:],
                                zt[:, j, i * 128:(i + 1) * 128],
                                yt[:, j, t0:t0 + 512],
                                start=(j == 0), stop=(j == DC - 1),
                            )
                        tmp = small.tile([128, 512], f32, tag="exp")
                        nc.scalar.activation(tmp[:], ps[:], EXP,
                                             bias=beta[:, i:i + 1],
                                             scale=SCALE)
                        nc.vector.tensor_mul(
                            pt[:, i], tmp[:], dag[:, i, t0:t0 + 512],
                        )

                    # U = PT^T @ [V | 1]; col 512 of U is l[t] = sum_s PT,
                    # landing per-partition in the second PSUM bank.
                    for tq in range(4):
                        t_ = th * 4 + tq
                        ub = psb.tile([128, 258], f32, tag="pb")
                        ua = psa.tile([128, 256], f32, tag="pa")
                        for i in range(TC):
                            lhsT = pt[:, i, tq * 128:(tq + 1) * 128]
                            nc.tensor.matmul(ub[:], lhsT, v[:, i, 256:514],
                                             start=(i == 0),
                                             stop=(i == TC - 1))
                            nc.tensor.matmul(ua[:], lhsT, v[:, i, 0:256],
                                             start=(i == 0),
                                             stop=(i == TC - 1))
                        lmax = small.tile([128, 1], f32, tag="lmax")
                        nc.vector.tensor_scalar_max(lmax[:], ub[:, 256:257],
                                                    1e-30)
                        linv = small.tile([128, 1], f32, tag="linv")
                        nc.vector.reciprocal(linv[:], lmax[:])
                        osb = small.tile([128, 512], f32, tag="osb")
                        nc.scalar.activation(osb[:, 256:512], ub[:, 0:256],
                                             COPY, bias=0.0, scale=linv[:])
                        nc.scalar.activation(osb[:, 0:256], ua[:],
                                             COPY, bias=0.0, scale=linv[:])
                        nc.scalar.dma_start(
                            out=Od[b, t_ * 128:(t_ + 1) * 128], in_=osb[:])

            if reps == 1:
                for b in range(BPC):
                    emit_batch(b)
            else:
                with tc.For_i(0, reps, 1):
                    for b in range(BPC):
                        emit_batch(b)

    nc.compile()
    return nc


def _get_nc():
    global _CACHED_NC
    if _CACHED_NC is None:
        _CACHED_NC = _build()
    return _CACHED_NC


def _prep_core_inputs(X, Y, dag, Wk, bk, Wq, bq, Wv, bv, mm_dt=None):
    """Build the 8 per-core input maps (host-side shard + transpose +
    weight fusion G = Wk Wq^T, g = Wk bq)."""
    import ml_dtypes
    dt = MM_DT if mm_dt is None else mm_dt
    mmnp = {bf16: ml_dtypes.bfloat16, f16: np.float16}.get(dt, np.float32)
    X = np.ascontiguousarray(np.asarray(X, dtype=np.float32))
    Y = np.ascontiguousarray(np.asarray(Y, dtype=np.float32))
    dag = np.ascontiguousarray(np.asarray(dag, dtype=np.float32))
    dag_r = dag.reshape(TC, 128, T).astype(ml_dtypes.bfloat16)
    Wk64 = np.asarray(Wk, np.float64)
    G = (Wk64 @ np.asarray(Wq, np.float64).T).astype(np.float32)
    g = (Wk64 @ np.asarray(bq, np.float64)).astype(np.float32)
    Wv_ext = np.concatenate(
        [np.asarray(Wv, np.float32), g.reshape(D, 1), g.reshape(D, 1)],
        axis=1)
    shared = {
        "dagr": dag_r,
        "Gr": G.reshape(DC, 128, D).astype(mmnp),
        "Wvr": Wv_ext.reshape(DC, 128, H + 2).astype(mmnp),
        "bvb": np.ascontiguousarray(
            np.broadcast_to(np.asarray(bv, np.float32), (128, H))),
    }
    in_maps = []
    for core in range(NCORES):
        sl = slice(core * BPC, (core + 1) * BPC)
        xt = np.ascontiguousarray(X[sl].transpose(0, 2, 1)).reshape(
            BPC, DC, 128, T).astype(mmnp, copy=False)
        yt = np.ascontiguousarray(Y[sl].transpose(0, 2, 1)).reshape(
            BPC, DC, 128, T).astype(mmnp, copy=False)
        in_maps.append({"XT": xt, "YT": yt, **shared})
    return in_maps


def kernel(X, Y, dag, Wk, bk, Wq, bq, Wv, bv):
    nc = _get_nc()
    in_maps = _prep_core_inputs(X, Y, dag, Wk, bk, Wq, bq, Wv, bv)
    last_err = None
    for _attempt in range(3):
        try:
            res = run_bass_kernel_spmd(nc, in_maps, list(range(NCORES)))
            break
        except Exception as e:  # transient NRT device errors -- retry
            last_err = e
    else:
        raise last_err
    return np.concatenate([res.results[i]["O"] for i in range(NCORES)],
                          axis=0)


# revision 5
# speedup vs baseline: 1.1185x; 1.1185x over previous
"""Sparse (DAG-masked) attention head on 8 Trainium2 NeuronCores.

Reference computation (per batch b of 64):
    K = X_b @ Wk + bk; Q = Y_b @ Wq + bq; V = X_b @ Wv + bv         [T=1024, H=512]
    S = Q @ K^T / sqrt(H); A = softmax(where(dag.T*S == 0, -inf, dag.T*S))
    O = A @ V   (fully-masked rows -> 0)

Strategy: data-parallel over batch (8 batches per core); weights + dag
replicated. All matmuls run in float32r (TF32-like, 1 cycle/row on PE,
~1e-4 relative error).

Key algebraic fusion: softmax over s is invariant to additive terms that
vary only in t, so
    S^T[s,t] = (X G Y^T)[s,t] + beta[s] + (t-only terms, cancel)
with G = Wk @ Wq^T and beta = X @ (Wk @ bq), both folded on the host.
This removes one of the three projections and both K/Q bias adds.

Scores are computed TRANSPOSED (ST[s,t] = Z @ Y^T with Z^T = G^T X^T) so
the softmax weights PT = dag * exp(ST/sqrt(H) + beta*scale) land directly
in the [s, t] layout needed as the stationary operand of the P @ V
matmul -- no on-chip transposes. Softmax skips max-subtraction (scores
are ~N(0,1); exp cannot overflow) and normalizes AFTER the V-matmul.

Two "free column" fusions keep every PE row a productive row:
  * beta[s] = X @ (Wk bq) is column 512 of the V projection: the rhs is
    [Wv | g] (514 wide, split 256+258 across two PSUM banks; f32r needs even free sizes, so the extra column is duplicated), so beta
    lands per-partition with no extra matmul and no scatter DMAs.
  * l[t] = sum_s PT[s,t] is column 512 of the U matmul: V is extended
    with a ones column, so the softmax denominator falls out of U's
    second PSUM bank already in per-partition layout.

Host-side prep: X/Y are transposed to [D, T] per batch (the PE contracts
over the partition dim).
"""

import numpy as np

import concourse.bass as bass
import concourse.mybir as mybir
import concourse.tile as tile
from concourse import bacc
from concourse.bass_utils import run_bass_kernel_spmd

B, T, D, H = 64, 1024, 512, 512
NCORES = 8
BPC = B // NCORES          # batches per core
DC = D // 128              # d chunks (4)
TC = T // 128              # t/s tiles (8)
SCALE = 1.0 / float(np.sqrt(H))

f32 = mybir.dt.float32
f32r = mybir.dt.float32r
bf16 = mybir.dt.bfloat16
f16 = mybir.dt.float16
EXP = mybir.ActivationFunctionType.Exp
COPY = mybir.ActivationFunctionType.Copy

_CACHED_NC = None

MM_DT = f32r               # matmul operand dtype: f32r (accurate) or bf16


def _build(reps=1, mm_dt=None):
    # reps>1 wraps the whole pipeline in a hardware loop that re-runs it on
    # the same data -- used only by the timing harness (wall-clock deltas
    # cancel the axon RPC overhead).
    dt = MM_DT if mm_dt is None else mm_dt
    nc = bacc.Bacc("TRN2", target_bir_lowering=False, debug=False,
                   num_devices=NCORES)

    XTd = nc.dram_tensor("XT", [BPC, DC, 128, T], dt, kind="ExternalInput").ap()
    YTd = nc.dram_tensor("YT", [BPC, DC, 128, T], dt, kind="ExternalInput").ap()
    DAGd = nc.dram_tensor("dagr", [TC, 128, T], bf16, kind="ExternalInput").ap()
    Gd = nc.dram_tensor("Gr", [DC, 128, D], dt, kind="ExternalInput").ap()
    Wvd = nc.dram_tensor("Wvr", [DC, 128, H + 2], dt, kind="ExternalInput").ap()
    Bvd = nc.dram_tensor("bvb", [128, H], f32, kind="ExternalInput").ap()
    Od = nc.dram_tensor("O", [BPC, T, H], f32, kind="ExternalOutput").ap()

    with tile.TileContext(nc) as tc:
        with (
            tc.tile_pool(name="const", bufs=1) as const,
            tc.tile_pool(name="data", bufs=1) as data,
            tc.tile_pool(name="data2", bufs=2) as data2,
            tc.tile_pool(name="pipe", bufs=2) as pipe,
            tc.tile_pool(name="small", bufs=3) as small,
            tc.tile_pool(name="psum3", bufs=4, space="PSUM") as psum3,
            tc.tile_pool(name="psa", bufs=2, space="PSUM") as psa,
            tc.tile_pool(name="psb", bufs=2, space="PSUM") as psb,
        ):
            # ---- resident tensors ----
            # Input streams split across the three DMA-capable queues
            # (SP / ACT / GPSIMD); batch-0 activations interleaved with G in
            # consumption order so the first matmul starts after ~1MB of DMA.
            gt = const.tile([128, DC, D], dt, tag="gt")
            wv = const.tile([128, DC, H + 2], dt, tag="wv")
            bvb = const.tile([128, H], f32, tag="bvb")
            dag = const.tile([128, TC, T], bf16, tag="dag")
            if reps == 1:
                xt0 = data2.tile([128, DC, T], dt, tag="xt")
                yt0 = data.tile([128, DC, T], dt, tag="yt")
                for c in range(DC):
                    nc.sync.dma_start(out=gt[:, c], in_=Gd[c])
                    nc.sync.dma_start(out=xt0[:, c], in_=XTd[0, c])
                for c in range(DC):
                    nc.scalar.dma_start(out=yt0[:, c], in_=YTd[0, c])
            else:
                xt0 = yt0 = None
                for c in range(DC):
                    nc.sync.dma_start(out=gt[:, c], in_=Gd[c])
            for c in range(DC):
                nc.gpsimd.dma_start(out=wv[:, c], in_=Wvd[c])
            nc.gpsimd.dma_start(out=bvb[:], in_=Bvd[:])
            for i in range(TC):
                nc.gpsimd.dma_start(out=dag[:, i], in_=DAGd[i])

            def emit_batch(b):
                # ---- load activations (transposed: [d, t]) ----
                # xt (used by ZT/V/beta, early) on SP, double-buffered;
                # yt (used by scores, later) on ACT.
                if b == 0 and xt0 is not None:
                    xt, yt = xt0, yt0
                else:
                    xt = data2.tile([128, DC, T], dt, tag="xt")
                    yt = data.tile([128, DC, T], dt, tag="yt")
                    for c in range(DC):
                        nc.sync.dma_start(out=xt[:, c], in_=XTd[b, c])
                        nc.scalar.dma_start(out=yt[:, c], in_=YTd[b, c])

                # ---- ZT[d', s] = G^T X^T: lhsT = G[d, d'_tile], rhs = XT ----
                zt = data.tile([128, DC, T], dt, tag="zt")
                for j in range(DC):
                    for hf in range(2):
                        ps = psum3.tile([128, 512], f32, tag="mm")
                        for c in range(DC):
                            nc.tensor.matmul(
                                ps[:],
                                gt[:, c, j * 128:(j + 1) * 128],
                                xt[:, c, hf * 512:(hf + 1) * 512],
                                start=(c == 0), stop=(c == DC - 1),
                            )
                        nc.scalar.activation(
                            zt[:, j, hf * 512:(hf + 1) * 512], ps[:],
                            COPY, bias=0.0, scale=1.0,
                        )
                # ---- V[s, h] (plus beta column, plus a ones column for l):
                # lhsT = XT[d, s_tile], rhs = [Wv | g] 513 wide split over two
                # PSUM banks; col 512 accumulates beta[s] = X @ (Wk bq).
                v = data.tile([128, TC, H + 2], dt, tag="v")
                beta = small.tile([128, TC], f32, tag="beta")
                for i in range(TC):
                    pa = psa.tile([128, 256], f32, tag="pa")
                    pb = psb.tile([128, 258], f32, tag="pb")
                    for c in range(DC):
                        lhsT = xt[:, c, i * 128:(i + 1) * 128]
                        nc.tensor.matmul(
                            pb[:], lhsT, wv[:, c, 256:514],
                            start=(c == 0), stop=(c == DC - 1),
                        )
                        nc.tensor.matmul(
                            pa[:], lhsT, wv[:, c, 0:256],
                            start=(c == 0), stop=(c == DC - 1),
                        )
                    nc.vector.tensor_add(v[:, i, 0:256], pa[:], bvb[:, 0:256])
                    nc.vector.tensor_add(v[:, i, 256:512], pb[:, 0:256],
                                         bvb[:, 256:512])
                    nc.vector.tensor_scalar_mul(
                        beta[:, i:i + 1], pb[:, 256:257], SCALE)
                    # ones column for the l fold in the U matmul
                    nc.scalar.activation(v[:, i, 512:514], pb[:, 256:258],
                                         COPY, bias=1.0, scale=0.0)

                # ---- scores + AV in two t-halves (AV of one half overlaps
                # the score matmuls of the next).
                for th in range(2):
                    t0 = th * 512
                    # PT[s, t] = dag * exp(ST*scale + beta), ST = Z @ Y^T
                    pt = pipe.tile([128, TC, 512], dt, tag="pt")
                    for i in range(TC):
                        ps = psum3.tile([128, 512], f32, tag="mm")
                        for j in range(DC):
                            nc.tensor.matmul(
                                ps[:],
                                zt[:, j, i * 128:(i + 1) * 128],
                                yt[:, j, t0:t0 + 512],
                                start=(j == 0), stop=(j == DC - 1),
                            )
                        tmp = small.tile([128, 512], f32, tag="exp")
                        nc.scalar.activation(tmp[:], ps[:], EXP,
                                             bias=beta[:, i:i + 1],
                                             scale=SCALE)
                        nc.vector.tensor_mul(
                            pt[:, i], tmp[:], dag[:, i, t0:t0 + 512],
                        )

                    # U = PT^T @ [V | 1]; col 512 of U is l[t] = sum_s PT,
                    # landing per-partition in the second PSUM bank.
                    # Software-pipelined: each tq's last (i=7) accumulation
                    # pair is deferred behind the next tq's first 7, hiding
                    # the exp/mask latency of pt[:, 7] behind real PE work.
                    def finish_u(tq, ub, ua):
                        t_ = th * 4 + tq
                        lhsT = pt[:, TC - 1, tq * 128:(tq + 1) * 128]
                        nc.tensor.matmul(ub[:], lhsT, v[:, TC - 1, 256:514],
                                         start=False, stop=True)
                        nc.tensor.matmul(ua[:], lhsT, v[:, TC - 1, 0:256],
                                         start=False, stop=True)
                        lmax = small.tile([128, 1], f32, tag="lmax")
                        nc.vector.tensor_scalar_max(lmax[:], ub[:, 256:257],
                                                    1e-30)
                        linv = small.tile([128, 1], f32, tag="linv")
                        nc.vector.reciprocal(linv[:], lmax[:])
                        osb = small.tile([128, 512], f32, tag="osb")
                        nc.scalar.activation(osb[:, 256:512], ub[:, 0:256],
                                             COPY, bias=0.0, scale=linv[:])
                        nc.scalar.activation(osb[:, 0:256], ua[:],
                                             COPY, bias=0.0, scale=linv[:])
                        nc.scalar.dma_start(
                            out=Od[b, t_ * 128:(t_ + 1) * 128], in_=osb[:])

                    pend = None
                    for tq in range(4):
                        ub = psb.tile([128, 258], f32, tag="pb")
                        ua = psa.tile([128, 256], f32, tag="pa")
                        for i in range(TC - 1):
                            lhsT = pt[:, i, tq * 128:(tq + 1) * 128]
                            nc.tensor.matmul(ub[:], lhsT, v[:, i, 256:514],
                                             start=(i == 0), stop=False)
                            nc.tensor.matmul(ua[:], lhsT, v[:, i, 0:256],
                                             start=(i == 0), stop=False)
                        if pend is not None:
                            finish_u(*pend)
                        pend = (tq, ub, ua)
                    finish_u(*pend)

            if reps == 1:
                for b in range(BPC):
                    emit_batch(b)
            else:
                with tc.For_i(0, reps, 1):
                    for b in range(BPC):
                        emit_batch(b)

    nc.compile()
    return nc


def _get_nc():
    global _CACHED_NC
    if _CACHED_NC is None:
        _CACHED_NC = _build()
    return _CACHED_NC


def _prep_core_inputs(X, Y, dag, Wk, bk, Wq, bq, Wv, bv, mm_dt=None):
    """Build the 8 per-core input maps (host-side shard + transpose +
    weight fusion G = Wk Wq^T, g = Wk bq)."""
    import ml_dtypes
    dt = MM_DT if mm_dt is None else mm_dt
    mmnp = {bf16: ml_dtypes.bfloat16, f16: np.float16}.get(dt, np.float32)
    X = np.ascontiguousarray(np.asarray(X, dtype=np.float32))
    Y = np.ascontiguousarray(np.asarray(Y, dtype=np.float32))
    dag = np.ascontiguousarray(np.asarray(dag, dtype=np.float32))
    dag_r = dag.reshape(TC, 128, T).astype(ml_dtypes.bfloat16)
    Wk64 = np.asarray(Wk, np.float64)
    G = (Wk64 @ np.asarray(Wq, np.float64).T).astype(np.float32)
    g = (Wk64 @ np.asarray(bq, np.float64)).astype(np.float32)
    Wv_ext = np.concatenate(
        [np.asarray(Wv, np.float32), g.reshape(D, 1), g.reshape(D, 1)],
        axis=1)
    shared = {
        "dagr": dag_r,
        "Gr": G.reshape(DC, 128, D).astype(mmnp),
        "Wvr": Wv_ext.reshape(DC, 128, H + 2).astype(mmnp),
        "bvb": np.ascontiguousarray(
            np.broadcast_to(np.asarray(bv, np.float32), (128, H))),
    }
    in_maps = []
    for core in range(NCORES):
        sl = slice(core * BPC, (core + 1) * BPC)
        xt = np.ascontiguousarray(X[sl].transpose(0, 2, 1)).reshape(
            BPC, DC, 128, T).astype(mmnp, copy=False)
        yt = np.ascontiguousarray(Y[sl].transpose(0, 2, 1)).reshape(
            BPC, DC, 128, T).astype(mmnp, copy=False)
        in_maps.append({"XT": xt, "YT": yt, **shared})
    return in_maps


def kernel(X, Y, dag, Wk, bk, Wq, bq, Wv, bv):
    nc = _get_nc()
    in_maps = _prep_core_inputs(X, Y, dag, Wk, bk, Wq, bq, Wv, bv)
    last_err = None
    for _attempt in range(3):
        try:
            res = run_bass_kernel_spmd(nc, in_maps, list(range(NCORES)))
            break
        except Exception as e:  # transient NRT device errors -- retry
            last_err = e
    else:
        raise last_err
    return np.concatenate([res.results[i]["O"] for i in range(NCORES)],
                          axis=0)


# revision 6
# speedup vs baseline: 1.4755x; 1.3192x over previous
"""Sparse (DAG-masked) attention head on 8 Trainium2 NeuronCores.

Reference computation (per batch b of 64):
    K = X_b @ Wk + bk; Q = Y_b @ Wq + bq; V = X_b @ Wv + bv         [T=1024, H=512]
    S = Q @ K^T / sqrt(H); A = softmax(where(dag.T*S == 0, -inf, dag.T*S))
    O = A @ V   (fully-masked rows -> 0)

Strategy: data-parallel over batch (8 batches per core); weights + dag
replicated. All matmuls run in float32r (TF32-like, 1 cycle/row on PE,
~1e-4 relative error).

Key algebraic fusion: softmax over s is invariant to additive terms that
vary only in t, so
    S^T[s,t] = (X G Y^T)[s,t] + beta[s] + (t-only terms, cancel)
with G = Wk @ Wq^T and beta = X @ (Wk @ bq), both folded on the host.
This removes one of the three projections and both K/Q bias adds.

Scores are computed TRANSPOSED (ST[s,t] = Z @ Y^T with Z^T = G^T X^T) so
the softmax weights PT = dag * exp(ST/sqrt(H) + beta*scale) land directly
in the [s, t] layout needed as the stationary operand of the P @ V
matmul -- no on-chip transposes. Softmax skips max-subtraction (scores
are ~N(0,1); exp cannot overflow) and normalizes AFTER the V-matmul.

Two "free column" fusions keep every PE row a productive row:
  * beta[s] = X @ (Wk bq) is column 512 of the V projection: the rhs is
    [Wv | g] (514 wide, split 256+258 across two PSUM banks; f32r needs even free sizes, so the extra column is duplicated), so beta
    lands per-partition with no extra matmul and no scatter DMAs.
  * l[t] = sum_s PT[s,t] is column 512 of the U matmul: V is extended
    with a ones column, so the softmax denominator falls out of U's
    second PSUM bank already in per-partition layout.

Host-side prep: X/Y are transposed to [D, T] per batch (the PE contracts
over the partition dim).
"""

import numpy as np

import concourse.bass as bass
import concourse.mybir as mybir
import concourse.tile as tile
from concourse import bacc
from concourse.bass_utils import run_bass_kernel_spmd

B, T, D, H = 64, 1024, 512, 512
NCORES = 8
BPC = B // NCORES          # batches per core
DC = D // 128              # d chunks (4)
TC = T // 128              # t/s tiles (8)
SCALE = 1.0 / float(np.sqrt(H))

f32 = mybir.dt.float32
f32r = mybir.dt.float32r
bf16 = mybir.dt.bfloat16
f16 = mybir.dt.float16
EXP = mybir.ActivationFunctionType.Exp
COPY = mybir.ActivationFunctionType.Copy

_CACHED_NC = None

MM_DT = f32r               # matmul operand dtype: f32r (accurate) or bf16


def _build(reps=1, mm_dt=None):
    # reps>1 wraps the whole pipeline in a hardware loop that re-runs it on
    # the same data -- used only by the timing harness (wall-clock deltas
    # cancel the axon RPC overhead).
    dt = MM_DT if mm_dt is None else mm_dt
    nc = bacc.Bacc("TRN2", target_bir_lowering=False, debug=False,
                   num_devices=NCORES)

    XTd = nc.dram_tensor("XT", [BPC, DC, 128, T], dt, kind="ExternalInput").ap()
    YTd = nc.dram_tensor("YT", [BPC, DC, 128, T], dt, kind="ExternalInput").ap()
    DAGd = nc.dram_tensor("dagr", [TC, 128, T], bf16, kind="ExternalInput").ap()
    Gd = nc.dram_tensor("Gr", [DC, 128, D], dt, kind="ExternalInput").ap()
    Wvd = nc.dram_tensor("Wvr", [DC, 128, H + 2], dt, kind="ExternalInput").ap()
    Bvd = nc.dram_tensor("bvb", [128, H], f32, kind="ExternalInput").ap()
    Od = nc.dram_tensor("O", [BPC, T, H], f32, kind="ExternalOutput").ap()

    with tile.TileContext(nc) as tc:
        with (
            tc.tile_pool(name="const", bufs=1) as const,
            tc.tile_pool(name="data", bufs=1) as data,
            tc.tile_pool(name="data2", bufs=2) as data2,
            tc.tile_pool(name="pipe", bufs=2) as pipe,
            tc.tile_pool(name="small", bufs=3) as small,
            tc.tile_pool(name="psum3", bufs=4, space="PSUM") as psum3,
            tc.tile_pool(name="psa", bufs=2, space="PSUM") as psa,
            tc.tile_pool(name="psb", bufs=2, space="PSUM") as psb,
        ):
            # ---- resident tensors ----
            # Input streams split across the three DMA-capable queues
            # (SP / ACT / GPSIMD); batch-0 activations interleaved with G in
            # consumption order so the first matmul starts after ~1MB of DMA.
            gt = const.tile([128, DC, D], dt, tag="gt")
            wv = const.tile([128, DC, H + 2], dt, tag="wv")
            bvb = const.tile([128, H], f32, tag="bvb")
            dag = const.tile([128, TC, T], bf16, tag="dag")
            if reps == 1:
                xt0 = data2.tile([128, DC, T], dt, tag="xt")
                yt0 = data.tile([128, DC, T], dt, tag="yt")
                for c in range(DC):
                    nc.sync.dma_start(out=gt[:, c], in_=Gd[c])
                    nc.sync.dma_start(out=xt0[:, c], in_=XTd[0, c])
                for c in range(DC):
                    nc.scalar.dma_start(out=yt0[:, c], in_=YTd[0, c])
            else:
                xt0 = yt0 = None
                for c in range(DC):
                    nc.sync.dma_start(out=gt[:, c], in_=Gd[c])
            for c in range(DC):
                nc.gpsimd.dma_start(out=wv[:, c], in_=Wvd[c])
            nc.gpsimd.dma_start(out=bvb[:], in_=Bvd[:])
            for i in range(TC):
                nc.gpsimd.dma_start(out=dag[:, i], in_=DAGd[i])

            def emit_batch(b):
                # ---- load activations (transposed: [d, t]) ----
                # xt (used by ZT/V/beta, early) on SP, double-buffered;
                # yt (used by scores, later) on ACT.
                if b == 0 and xt0 is not None:
                    xt, yt = xt0, yt0
                else:
                    xt = data2.tile([128, DC, T], dt, tag="xt")
                    yt = data.tile([128, DC, T], dt, tag="yt")
                    for c in range(DC):
                        nc.sync.dma_start(out=xt[:, c], in_=XTd[b, c])
                        nc.scalar.dma_start(out=yt[:, c], in_=YTd[b, c])

                # ---- ZT[d', s] = G^T X^T: lhsT = G[d, d'_tile], rhs = XT ----
                zt = data.tile([128, DC, T], dt, tag="zt")
                for j in range(DC):
                    for hf in range(2):
                        ps = psum3.tile([128, 512], f32, tag="mm")
                        for c in range(DC):
                            nc.tensor.matmul(
                                ps[:],
                                gt[:, c, j * 128:(j + 1) * 128],
                                xt[:, c, hf * 512:(hf + 1) * 512],
                                start=(c == 0), stop=(c == DC - 1),
                            )
                        nc.scalar.activation(
                            zt[:, j, hf * 512:(hf + 1) * 512], ps[:],
                            COPY, bias=0.0, scale=1.0,
                        )
                # ---- V[s, h] (plus beta column, plus a ones column for l):
                # lhsT = XT[d, s_tile], rhs = [Wv | g] 513 wide split over two
                # PSUM banks; col 512 accumulates beta[s] = X @ (Wk bq).
                v = data.tile([128, TC, H + 2], dt, tag="v")
                beta = small.tile([128, TC], f32, tag="beta")
                for i in range(TC):
                    pa = psa.tile([128, 256], f32, tag="pa")
                    pb = psb.tile([128, 258], f32, tag="pb")
                    for c in range(DC):
                        lhsT = xt[:, c, i * 128:(i + 1) * 128]
                        nc.tensor.matmul(
                            pb[:], lhsT, wv[:, c, 256:514],
                            start=(c == 0), stop=(c == DC - 1),
                        )
                        nc.tensor.matmul(
                            pa[:], lhsT, wv[:, c, 0:256],
                            start=(c == 0), stop=(c == DC - 1),
                        )
                    nc.vector.tensor_add(v[:, i, 0:256], pa[:], bvb[:, 0:256])
                    nc.vector.tensor_add(v[:, i, 256:512], pb[:, 0:256],
                                         bvb[:, 256:512])
                    nc.vector.tensor_scalar_mul(
                        beta[:, i:i + 1], pb[:, 256:257], SCALE)
                    # ones column for the l fold in the U matmul
                    nc.scalar.activation(v[:, i, 512:514], pb[:, 256:258],
                                         COPY, bias=1.0, scale=0.0)

                # ---- scores + AV in two t-halves (AV of one half overlaps
                # the score matmuls of the next).
                for th in range(2):
                    t0 = th * 512
                    # PT[s, t] = dag * exp(ST*scale + beta), ST = Z @ Y^T
                    pt = pipe.tile([128, TC, 512], dt, tag="pt")
                    for i in range(TC):
                        ps = psum3.tile([128, 512], f32, tag="mm")
                        for j in range(DC):
                            nc.tensor.matmul(
                                ps[:],
                                zt[:, j, i * 128:(i + 1) * 128],
                                yt[:, j, t0:t0 + 512],
                                start=(j == 0), stop=(j == DC - 1),
                            )
                        tmp = small.tile([128, 512], f32, tag="exp")
                        nc.scalar.activation(tmp[:], ps[:], EXP,
                                             bias=beta[:, i:i + 1],
                                             scale=SCALE)
                        nc.vector.tensor_mul(
                            pt[:, i], tmp[:], dag[:, i, t0:t0 + 512],
                        )

                    # U = PT^T @ [V | 1]; col 512 of U is l[t] = sum_s PT,
                    # landing per-partition in the second PSUM bank.
                    for tq in range(4):
                        t_ = th * 4 + tq
                        ub = psb.tile([128, 258], f32, tag="pb")
                        ua = psa.tile([128, 256], f32, tag="pa")
                        for i in range(TC):
                            lhsT = pt[:, i, tq * 128:(tq + 1) * 128]
                            nc.tensor.matmul(ub[:], lhsT, v[:, i, 256:514],
                                             start=(i == 0),
                                             stop=(i == TC - 1))
                            nc.tensor.matmul(ua[:], lhsT, v[:, i, 0:256],
                                             start=(i == 0),
                                             stop=(i == TC - 1))
                        lmax = small.tile([128, 1], f32, tag="lmax")
                        nc.vector.tensor_scalar_max(lmax[:], ub[:, 256:257],
                                                    1e-30)
                        linv = small.tile([128, 1], f32, tag="linv")
                        nc.vector.reciprocal(linv[:], lmax[:])
                        osb = small.tile([128, 512], f32, tag="osb")
                        nc.scalar.activation(osb[:, 256:512], ub[:, 0:256],
                                             COPY, bias=0.0, scale=linv[:])
                        nc.scalar.activation(osb[:, 0:256], ua[:],
                                             COPY, bias=0.0, scale=linv[:])
                        nc.scalar.dma_start(
                            out=Od[b, t_ * 128:(t_ + 1) * 128], in_=osb[:])

            if reps == 1:
                for b in range(BPC):
                    emit_batch(b)
            else:
                with tc.For_i(0, reps, 1):
                    for b in range(BPC):
                        emit_batch(b)

    nc.compile()
    return nc


def _get_nc():
    global _CACHED_NC
    if _CACHED_NC is None:
        _CACHED_NC = _build()
    return _CACHED_NC


def _prep_core_inputs(X, Y, dag, Wk, bk, Wq, bq, Wv, bv, mm_dt=None):
    """Build the 8 per-core input maps (host-side shard + transpose +
    weight fusion G = Wk Wq^T, g = Wk bq)."""
    import ml_dtypes
    dt = MM_DT if mm_dt is None else mm_dt
    mmnp = {bf16: ml_dtypes.bfloat16, f16: np.float16}.get(dt, np.float32)
    X = np.ascontiguousarray(np.asarray(X, dtype=np.float32))
    Y = np.ascontiguousarray(np.asarray(Y, dtype=np.float32))
    dag = np.ascontiguousarray(np.asarray(dag, dtype=np.float32))
    dag_r = dag.reshape(TC, 128, T).astype(ml_dtypes.bfloat16)
    Wk64 = np.asarray(Wk, np.float64)
    G = (Wk64 @ np.asarray(Wq, np.float64).T).astype(np.float32)
    g = (Wk64 @ np.asarray(bq, np.float64)).astype(np.float32)
    Wv_ext = np.concatenate(
        [np.asarray(Wv, np.float32), g.reshape(D, 1), g.reshape(D, 1)],
        axis=1)
    shared = {
        "dagr": dag_r,
        "Gr": G.reshape(DC, 128, D).astype(mmnp),
        "Wvr": Wv_ext.reshape(DC, 128, H + 2).astype(mmnp),
        "bvb": np.ascontiguousarray(
            np.broadcast_to(np.asarray(bv, np.float32), (128, H))),
    }
    in_maps = []
    for core in range(NCORES):
        sl = slice(core * BPC, (core + 1) * BPC)
        xt = np.ascontiguousarray(X[sl].transpose(0, 2, 1)).reshape(
            BPC, DC, 128, T).astype(mmnp, copy=False)
        yt = np.ascontiguousarray(Y[sl].transpose(0, 2, 1)).reshape(
            BPC, DC, 128, T).astype(mmnp, copy=False)
        in_maps.append({"XT": xt, "YT": yt, **shared})
    return in_maps


def kernel(X, Y, dag, Wk, bk, Wq, bq, Wv, bv):
    nc = _get_nc()
    in_maps = _prep_core_inputs(X, Y, dag, Wk, bk, Wq, bq, Wv, bv)
    last_err = None
    for _attempt in range(3):
        try:
            res = run_bass_kernel_spmd(nc, in_maps, list(range(NCORES)))
            break
        except Exception as e:  # transient NRT device errors -- retry
            last_err = e
    else:
        raise last_err
    return np.concatenate([res.results[i]["O"] for i in range(NCORES)],
                          axis=0)
